# revision 10
# baseline (speedup 1.0000x reference)
"""Multi-head attention (B=4, G=2048, C=1024, H=16) on 8 TRN2 NeuronCores.

Sharding: (batch x head-half). Core c handles batch c//2 and an 8-head
slice (c%2); the host sums core pairs and adds the bias.

v2 design, built around the TimelineSim cost model (matmul cost =
out-free-size x cycles-per-row, independent of K/M; fp8e4 DoubleRow =
0.5 cycles/row contracting 2 k-tiles):

  - qkv projections: compensated fp8 DoubleRow. Host uploads
    x8=f8(8x), xr=f8(8x-x8), W8=f8(64W), Wr=f8(64W-W8); the device
    computes W8'x8 + W8'xr + Wr'x8 (12 DoubleRow matmuls per output
    tile vs 8 bf16) at ~1e-3 relative error.
  - scores: one fp8 DoubleRow per (head, key-block, 512-query) with
    in-instruction q-residual compensation: k-tiles (k8,k8) x (q8,qr)
    = k8.(q8+qr). Only k's raw fp8 quantization (~1%) survives.
  - softmax exp: ScalarE activation(Exp) for most score tiles; the
    rest are egressed PSUM->SBUF by DVE and exponentiated on GPSIMD
    via tensor_tensor(pow) with base exp(1/512) (scores carry a 512x
    scale from the fp8 power-of-2 scaling).
  - AV: transposed formulation. out[q,65] += ex[k,q]^T v[k,65] per
    (head, 128-query block, key block) -- full 128-partition output
    (the baseline's [65,1024] orientation wasted half the PE). The
    65th ones-lane of v gives the softmax denominator per query ON
    THE PARTITION, so normalization is a per-partition reciprocal +
    tensor_scalar multiply; no cross-partition broadcast needed.
  - head-merge transpose o[q,chan] -> oT[chan,q] via dma_start_transpose
    (DMA xbar, 14ns/32x32 tile), freeing PE and DVE.
  - output projection: bf16 as baseline, f16 store.

The stream is emitted flat in deadline order (baseline-style): scores
for unit u+1 interleave kb-wise with AV of unit u; q/k/v projection
blocks are pinned just before first use; output-projection halves
drip in once their query half's heads are done.
"""

from contextlib import ExitStack

import numpy as np

import concourse.bass as bass
import concourse.tile as tile
from concourse import mybir
from concourse.bass_utils import run_bass_kernel_spmd
from concourse.vector_clock import ScopedClock, VectorClock
from concourse.tile_sem_assignment import N_PROCS

F32 = mybir.dt.float32
F16 = mybir.dt.float16
BF16 = mybir.dt.bfloat16
F8 = mybir.dt.float8e4

B, G, C, H = 4, 2048, 1024, 16
N_CORES = 8
H_LOC = H // 2          # heads per core
O_LOC = H_LOC * 64      # output channels per core
D = 64                  # head dim
CC = C // 128           # contraction blocks for projections
KC = G // 128           # key-token blocks
T = H_LOC // 2          # head pairs per core
NU = 2 * H_LOC          # units: (query-half, head)

SX = 8.0                # x fp8 scale
SW = 64.0               # W fp8 scale
SQK = 2.0 ** -6         # psum(512*q) -> q8 at scale 8
SV = 2.0 ** -9          # psum(512*v) -> v
EXP_SCALE = 1.0 / 512.0  # score psum carries 64x; logits are /8
DR = mybir.MatmulPerfMode.DoubleRow

# kb values (0..15) whose exp runs on GPSIMD via DVE egress + pow
# (interleaved with ScalarE tiles so consecutive sc-ring tiles never
# serialize on one consumer engine)
POOL_KB = {1, 3, 5, 7, 9, 11, 13}
WARM_N = 34             # dummy matmuls bridging the input-DMA front
DRIP_RATE = 0.12


class SplitDrainTileContext(tile.TileContext):
    """Tail drain limited to one sync wait per instruction.

    This environment's walrus rejects >1 sync wait per instruction, so
    wait on each outstanding proc tick with its own NOP first and emit
    the drain bare.
    """

    def _drain_and_barrier(self, tick_clock, wait_clock):
        g = tick_clock.global_clock
        for p in range(N_PROCS):
            if g[p] > 0:
                nop = self.nc.sync.nop(nofuse=True)
                partial = VectorClock([g[q] if q == p else 0 for q in range(N_PROCS)])
                wait_clock.add_sem_waits(nop.ins, ScopedClock({None: partial}))
        self.nc.sync.drain()
        self.nc.all_engine_barrier()
        assert self.sems is not None
        popped = self.nc._tile_sem_poison_stack.pop()
        assert popped is self._sem_poison
        self.nc.clear_and_free_semaphores(list(self.sems.allocated().values()))
        self.nc.all_engine_barrier()


def split_multi_waits(nc):
    """Hoist extra sync waits onto NOPs before each offending instruction
    (this walrus accepts at most one sync wait per instruction)."""
    n_split = 0
    for f in nc.m.functions:
        for bb in f.blocks:
            insts = bb.instructions
            out = []
            for inst in insts:
                si = inst.sync_info
                waits = list(si.on_wait) if si and si.on_wait else []
                if len(waits) > 1:
                    for w in waits[:-1]:
                        nop = mybir.InstNoOp(
                            name=f"{inst.name}_w{n_split}",
                            engine=inst.engine,
                            ins=[],
                            outs=[],
                            sync_info=mybir.SyncInfo(on_wait=[w], on_update=[]),
                        )
                        out.append(nop)
                        n_split += 1
                    inst.sync_info = mybir.SyncInfo(
                        on_wait=[waits[-1]],
                        on_update=list(si.on_update) if si.on_update else [],
                    )
                out.append(inst)
            if len(out) != len(insts):
                bb.instructions[:] = out
    return n_split


def build_program():
    nc = bass.Bass()
    x8_d = nc.declare_dram_parameter("x8", [C, G], F8, isOutput=False)
    xr_d = nc.declare_dram_parameter("xr", [C, G], F8, isOutput=False)
    wq8_d = nc.declare_dram_parameter("wq8", [C, O_LOC], F8, isOutput=False)
    wqr_d = nc.declare_dram_parameter("wqr", [C, O_LOC], F8, isOutput=False)
    wk8_d = nc.declare_dram_parameter("wk8", [C, O_LOC], F8, isOutput=False)
    wkr_d = nc.declare_dram_parameter("wkr", [C, O_LOC], F8, isOutput=False)
    wv8_d = nc.declare_dram_parameter("wv8", [C, O_LOC], F8, isOutput=False)
    wvr_d = nc.declare_dram_parameter("wvr", [C, O_LOC], F8, isOutput=False)
    wp_d = nc.declare_dram_parameter("wp", [O_LOC, C], BF16, isOutput=False)
    out_p = nc.declare_dram_parameter("out_p", [G, C], F16, isOutput=True)

    with SplitDrainTileContext(nc) as tc, ExitStack() as ctx:
        persist = ctx.enter_context(tc.tile_pool(name="persist", bufs=1))
        x8 = persist.tile([128, CC, G], F8, name="x8s", tag="x8s")
        xr = persist.tile([128, CC, G], F8, name="xrs", tag="xrs")
        wq8 = persist.tile([128, CC, O_LOC], F8, name="wq8s", tag="wq8s")
        wqr = persist.tile([128, CC, O_LOC], F8, name="wqrs", tag="wqrs")
        wk8 = persist.tile([128, CC, O_LOC], F8, name="wk8s", tag="wk8s")
        wkr = persist.tile([128, CC, O_LOC], F8, name="wkrs", tag="wkrs")
        wv8 = persist.tile([128, CC, O_LOC], F8, name="wv8s", tag="wv8s")
        wvr = persist.tile([128, CC, O_LOC], F8, name="wvrs", tag="wvrs")
        wp_sb = persist.tile([128, T, C], BF16, name="wp_sb", tag="wp_sb")
        qT8 = [persist.tile([128, 2, G], F8, name=f"qT8_{t}", tag=f"qT8_{t}")
               for t in range(T)]
        kT8 = [persist.tile([128, 2, G], F8, name=f"kT8_{t}", tag=f"kT8_{t}")
               for t in range(T)]
        v_sb = persist.tile([128, KC, H_LOC, 65], BF16, name="v_sb", tag="v_sb")
        oT_t = [persist.tile([128, G], BF16, name=f"oT{t}", tag=f"oT{t}")
                for t in range(T)]
        cpow = persist.tile([128, 1024], F32, name="cpow", tag="cpow")

        # Input loads (one queue, dependency-priority order; transfers
        # serialize on the DMA fabric).
        def ld(dst, src, rearr):
            nc.sync.dma_start(out=dst[:], in_=src.rearrange(rearr, p=128))
        RX = "(cc p) g -> p cc g"
        RW = "(cc p) o -> p cc o"
        nc.sync.dma_start(out=x8[:, :, 0:1024],
                          in_=x8_d[:, 0:1024].rearrange(RX, p=128))
        ld(wq8, wq8_d, RW)
        ld(wqr, wqr_d, RW)
        nc.sync.dma_start(out=xr[:, :, 0:1024],
                          in_=xr_d[:, 0:1024].rearrange(RX, p=128))
        ld(wk8, wk8_d, RW)
        ld(wkr, wkr_d, RW)
        nc.sync.dma_start(out=x8[:, :, 1024:2048],
                          in_=x8_d[:, 1024:2048].rearrange(RX, p=128))
        nc.sync.dma_start(out=xr[:, :, 1024:2048],
                          in_=xr_d[:, 1024:2048].rearrange(RX, p=128))
        ld(wv8, wv8_d, RW)
        ld(wvr, wvr_d, RW)
        ld(wp_sb, wp_d, "(ct p) o -> p ct o")

        # Constants + warm-up (PE clock ramp while DMAs land).
        wmpool = ctx.enter_context(tc.tile_pool(name="warm", bufs=1))
        wz = wmpool.tile([128, 512], BF16, name="wz", tag="wz")
        nc.vector.memset(wz[:], 0.0)
        ones_t = wmpool.tile([128, KC, H_LOC, 1], BF16, name="ones_t", tag="ones")
        nc.vector.memset(ones_t[:], 1.0)
        nc.vector.tensor_copy(out=v_sb[:, :, :, 64:65], in_=ones_t[:])
        nc.vector.memset(cpow[:], float(np.exp(EXP_SCALE)))

        scps = ctx.enter_context(tc.tile_pool(name="sc", bufs=2, space="PSUM"))
        avps = ctx.enter_context(tc.tile_pool(name="avp", bufs=2, space="PSUM"))
        fillps = ctx.enter_context(tc.tile_pool(name="fillp", bufs=2, space="PSUM"))
        expool = ctx.enter_context(tc.tile_pool(name="ex", bufs=24))
        scspool = ctx.enter_context(tc.tile_pool(name="scs", bufs=3))
        opool = ctx.enter_context(tc.tile_pool(name="ostg", bufs=2))
        rpool = ctx.enter_context(tc.tile_pool(name="rcp", bufs=4))
        stpool = ctx.enter_context(tc.tile_pool(name="st", bufs=4))

        def warm(n):
            wps = fillps.tile([128, 512], F32, name="wps", tag="fill")
            for _ in range(n):
                nc.tensor.matmul(wps[:], wz[:, 0:128], wz[:], start=True, stop=True)

        warm(WARM_N)

        written = set()

        def mark(*key):
            written.add(key)

        def need(*key):
            assert key in written, f"use-before-def: {key}"

        # ---- projection building blocks (compensated fp8 DoubleRow) ----
        TERMS_QK = ((0, 0), (0, 1), (1, 0))  # (w residual?, x residual?)

        def v_block(gc):
            ps = fillps.tile([128, 512], F32, name="ps_v", tag="fill")
            first, last = (0, 0, 0), (1, 0, CC // 2 - 1)
            for (rw, rx) in TERMS_QK:
                wv_ = wvr if rw else wv8
                x_ = xr if rx else x8
                for j in range(CC // 2):
                    nc.tensor.matmul(
                        ps[:],
                        x_[:, 2 * j:2 * j + 2, gc * 128:(gc + 1) * 128],
                        wv_[:, 2 * j:2 * j + 2, :],
                        start=((rw, rx, j) == first), stop=((rw, rx, j) == last),
                        perf_mode=DR,
                    )
            nc.scalar.mul(v_sb[:, gc, :, 0:64], ps[:], SV)
            mark("v", gc)

        def qk_quarter(t, which, gh, q4):
            g0 = gh * 1024 + q4 * 256
            ps = fillps.tile([128, 256], F32, name="ps_qk", tag="fill")
            w8_, wr_ = (wq8, wqr) if which == 0 else (wk8, wkr)
            first, last = (0, 0, 0), (1, 0, CC // 2 - 1)
            for (rw, rx) in TERMS_QK:
                w_ = wr_ if rw else w8_
                x_ = xr if rx else x8
                for j in range(CC // 2):
                    nc.tensor.matmul(
                        ps[:],
                        w_[:, 2 * j:2 * j + 2, t * 128:(t + 1) * 128],
                        x_[:, 2 * j:2 * j + 2, g0:g0 + 256],
                        start=((rw, rx, j) == first), stop=((rw, rx, j) == last),
                        perf_mode=DR,
                    )
            dst = qT8[t] if which == 0 else kT8[t]
            if which == 0:
                nc.vector.tensor_scalar_mul(dst[:, 0, g0:g0 + 256], ps[:], SQK)
                # q residual at the same scale: qr = psum*2^-6 - q8
                nc.vector.scalar_tensor_tensor(
                    out=dst[:, 1, g0:g0 + 256], in0=ps[:], scalar=SQK,
                    in1=dst[:, 0, g0:g0 + 256],
                    op0=mybir.AluOpType.mult, op1=mybir.AluOpType.subtract,
                )
            else:
                nc.scalar.mul(dst[:, 0, g0:g0 + 256], ps[:], SQK)
                # k8 duplicated into the second k-tile slot (GPSIMD, SBUF only)
                nc.gpsimd.tensor_copy(out=dst[:, 1, g0:g0 + 256],
                                      in_=dst[:, 0, g0:g0 + 256])
            mark("qk", which, t, gh, q4)

        # ---- attention building blocks ----
        def sc_op(u, kb):
            h, qh = u % H_LOC, u // H_LOC
            t, base = h // 2, (h % 2) * 64
            for q4 in range(4):
                need("qk", 0, t, qh, q4)
            need("qk", 1, t, kb // 8, (kb % 8) // 2)
            sc = scps.tile([128, 1024], F32, name="sc", tag="sc")
            for z in range(2):
                nc.tensor.matmul(
                    sc[:, z * 512:(z + 1) * 512],
                    kT8[t][base:base + D, :, kb * 128:(kb + 1) * 128],
                    qT8[t][base:base + D, :,
                           qh * 1024 + z * 512: qh * 1024 + (z + 1) * 512],
                    start=True, stop=True, perf_mode=DR,
                )
            ex = expool.tile([128, 1024], BF16, name="ex", tag="ex")
            if kb in POOL_KB:
                scs = scspool.tile([128, 1024], F32, name="scs", tag="scs")
                nc.vector.tensor_copy(out=scs[:], in_=sc[:])
                nc.gpsimd.tensor_tensor(out=ex[:], in0=cpow[:], in1=scs[:],
                                        op=mybir.AluOpType.pow)
            else:
                nc.scalar.activation(
                    out=ex[:], in_=sc[:],
                    func=mybir.ActivationFunctionType.Exp, scale=EXP_SCALE,
                )
            return ex

        def av_op(u, kc, pos, av2, ex):
            # pos = index of this kc in the slot's processing order (0..15);
            # psum accumulation is order-independent, so Pool-exp'd kbs are
            # consumed last for extra exp slack.
            h = u % H_LOC
            need("v", kc)
            for qb in range(8):
                nc.tensor.matmul(
                    av2[qb // 4][:, qb % 4, :],
                    ex[:, qb * 128:(qb + 1) * 128],
                    v_sb[:, kc, h, :],
                    start=(pos == 0 and qb % 4 == 0),
                    stop=(pos == KC - 1 and qb % 4 == 3),
                    skip_group_check=True,
                )

        ostg = {}

        def norm_op(u, av2):
            h, qh = u % H_LOC, u // H_LOC
            t, base = h // 2, (h % 2) * 64
            if (t, qh) not in ostg:
                ostg[(t, qh)] = opool.tile([128, 8, 128], BF16,
                                           name="ostg", tag="ostg")
            o = ostg[(t, qh)]
            for half in range(2):
                rcp = rpool.tile([128, 4], F32, name="rcp", tag="rcp")
                nc.vector.reciprocal(out=rcp[:], in_=av2[half][:, :, 64])
                for idx in range(4):
                    qb = half * 4 + idx
                    nc.vector.tensor_scalar(
                        out=o[:, qb, base:base + 64],
                        in0=av2[half][:, idx, 0:64],
                        scalar1=rcp[:, idx:idx + 1], scalar2=None,
                        op0=mybir.AluOpType.mult,
                    )
            mark("o", t, qh)

        def transpose_op(t, qh, qb):
            need("o", t, qh)
            nc.sync.dma_start_transpose(
                oT_t[t][:, qh * 1024 + qb * 128: qh * 1024 + (qb + 1) * 128],
                ostg[(t, qh)][:, qb, :])
            mark("oT", t, qh, qb)

        def proj_half(gc, z):
            qh, qb = gc // 8, gc % 8
            for ct in range(T):
                need("oT", ct, qh, qb)
            po = fillps.tile([128, 512], F32, name="po", tag="fill")
            for ct in range(T):
                nc.tensor.matmul(
                    po[:],
                    oT_t[ct][:, gc * 128:(gc + 1) * 128],
                    wp_sb[:, ct, z * 512:(z + 1) * 512],
                    start=(ct == 0), stop=(ct == T - 1),
                )
            st = stpool.tile([128, 512], F16, name="st", tag="st")
            nc.scalar.copy(out=st[:], in_=po[:])
            eng = (nc.sync, nc.scalar)[(2 * gc + z) % 2]
            eng.dma_start(
                out=out_p[gc * 128:(gc + 1) * 128, z * 512:(z + 1) * 512],
                in_=st[:],
            )

        # ---- the flat stream ----
        # Absolute step s: sc(u, kb) at s = 16u + kb; av(u, kc) at
        # s = 16(u+1) + kc; norm(u) at the end of step 16(u+1)+15.
        pinned = {}

        def pin(step, fn):
            pinned.setdefault(max(0, step), []).append(fn)

        # q quarters: needed by sc(u = qh*8 + 2t + (0|1), kb=0).
        for t in range(T):
            for qh in range(2):
                for q4 in range(4):
                    pin(16 * (qh * 8 + 2 * t) - 8 + q4,
                        lambda t=t, qh=qh, q4=q4: qk_quarter(t, 0, qh, q4))
        # k quarters: quarter (gh, q4) covers kb = gh*8 + 2*q4 (+1); first
        # used by unit 2t at s = 32t + kb.
        for t in range(T):
            for gh in range(2):
                for q4 in range(4):
                    pin(32 * t + gh * 8 + 2 * q4 - 6,
                        lambda t=t, gh=gh, q4=q4: qk_quarter(t, 1, gh, q4))
        # v blocks: v(kc) needed by av(0, kc) at s = 16 + pos(kc) in the
        # reordered AV consumption sequence.
        AV_ORDER0 = [kb for kb in range(KC) if kb not in POOL_KB] + sorted(POOL_KB)
        for kc in range(KC):
            pin(14 + AV_ORDER0.index(kc), lambda kc=kc: v_block(kc))
        # proj halves for query-half 0: transposes done after norm(u=7)
        # (end of s=143); drip them across slots 9..16.
        for i, (gc, z) in enumerate((gc, z) for gc in range(8) for z in range(2)):
            pin(150 + 7 * i, lambda gc=gc, z=z: proj_half(gc, z))

        filler = []
        filler_i = 0
        filler_budget = 0.0

        def drip(budget):
            nonlocal filler_i, filler_budget
            if filler_i >= len(filler):
                filler_budget = 0.0
                return
            filler_budget = min(filler_budget + budget, 2.0)
            while filler_budget >= 1.0 and filler_i < len(filler):
                gc, z = filler[filler_i]
                filler_i += 1
                filler_budget -= 1.0
                proj_half(gc, z)

        exs = {}
        av_tiles = {}

        def issue_sc(u, kb):
            exs[(u, kb)] = sc_op(u, kb)

        # AV consumption order: ScalarE-exp'd kbs first, Pool-exp'd last.
        AV_ORDER = [kb for kb in range(KC) if kb not in POOL_KB] + sorted(POOL_KB)

        n_steps = 16 * (NU + 1)
        for s in range(n_steps):
            slot, kb = divmod(s, 16)
            u_sc = slot            # scores unit (one slot ahead)
            u_av = slot - 1        # AV unit
            for op in pinned.pop(s, ()):
                op()
            if u_sc < NU:
                issue_sc(u_sc, kb)
            if u_av >= 0:
                if kb == 0:
                    av_tiles[u_av] = [
                        avps.tile([128, 4, 65], F32, name="av", tag="av")
                        for _ in range(2)]
                kc = AV_ORDER[kb]
                av_op(u_av, kc, kb, av_tiles[u_av], exs.pop((u_av, kc)))
                if s >= 160:
                    drip(DRIP_RATE)
                if kb == 15:
                    norm_op(u_av, av_tiles.pop(u_av))
                    h, qh = u_av % H_LOC, u_av // H_LOC
                    if h % 2 == 1:
                        for qb in range(8):
                            transpose_op(h // 2, qh, qb)
                    if u_av == 7:
                        pass  # qh0 proj halves are pinned above
                    if u_av == NU - 1:
                        filler.extend((gc, z) for gc in range(8, KC)
                                      for z in range(2))
        for op_step in sorted(pinned):
            for op in pinned.pop(op_step, ()):
                op()
        while filler_i < len(filler):
            gc, z = filler[filler_i]
            filler_i += 1
            proj_half(gc, z)

    split_multi_waits(nc)
    return nc


_CACHE = {}

_F8_NP = mybir.dt.np(F8)
_BF16_NP = mybir.dt.np(BF16)


def _split_f8(a, scale):
    hi = (a * scale).astype(_F8_NP)
    lo = (a * scale - hi.astype(np.float32)).astype(_F8_NP)
    return hi, lo


def make_in_maps(x, Wq, Wk, Wv, Wp):
    x = np.asarray(x, dtype=np.float32)
    WqT = np.asarray(Wq, dtype=np.float32).T
    WkT = np.asarray(Wk, dtype=np.float32).T
    WvT = np.asarray(Wv, dtype=np.float32).T
    WpT = np.asarray(Wp, dtype=np.float32).T
    in_maps = []
    for core in range(N_CORES):
        b, s = core // 2, core % 2
        osl = slice(s * O_LOC, (s + 1) * O_LOC)
        x8, xr = _split_f8(np.ascontiguousarray(x[b].T), SX)
        wq8, wqr = _split_f8(np.ascontiguousarray(WqT[:, osl]), SW)
        wk8, wkr = _split_f8(np.ascontiguousarray(WkT[:, osl]), SW)
        wv8, wvr = _split_f8(np.ascontiguousarray(WvT[:, osl]), SW)
        in_maps.append({
            "x8": x8, "xr": xr,
            "wq8": wq8, "wqr": wqr,
            "wk8": wk8, "wkr": wkr,
            "wv8": wv8, "wvr": wvr,
            "wp": np.ascontiguousarray(WpT[osl, :]).astype(_BF16_NP),
        })
    return in_maps


def kernel(x, Wq, Wk, Wv, Wp, bp):
    in_maps = make_in_maps(x, Wq, Wk, Wv, Wp)
    if "nc" not in _CACHE:
        _CACHE["nc"] = build_program()
    res = run_bass_kernel_spmd(_CACHE["nc"], in_maps, list(range(N_CORES)))
    out = np.zeros((B, G, C), np.float32)
    bp = np.asarray(bp, dtype=np.float32)
    for b in range(B):
        out[b] = (res.results[2 * b]["out_p"].astype(np.float32)
                  + res.results[2 * b + 1]["out_p"].astype(np.float32) + bp)
    return out


# revision 13
# speedup vs baseline: 1.0166x; 1.0166x over previous
"""Multi-head attention (B=4, G=2048, C=1024, H=16) on 8 TRN2 NeuronCores.

Sharding: (batch x head-half). Core c handles batch c//2 and an 8-head
slice (c%2); the host sums core pairs and adds the bias.

v2 design, built around the TimelineSim cost model (matmul cost =
out-free-size x cycles-per-row, independent of K/M; fp8e4 DoubleRow =
0.5 cycles/row contracting 2 k-tiles):

  - qkv projections: compensated fp8 DoubleRow. Host uploads
    x8=f8(8x), xr=f8(8x-x8), W8=f8(64W), Wr=f8(64W-W8); the device
    computes W8'x8 + W8'xr + Wr'x8 (12 DoubleRow matmuls per output
    tile vs 8 bf16) at ~1e-3 relative error.
  - scores: one fp8 DoubleRow per (head, key-block, 512-query) with
    in-instruction q-residual compensation: k-tiles (k8,k8) x (q8,qr)
    = k8.(q8+qr). Only k's raw fp8 quantization (~1%) survives.
  - softmax exp: ScalarE activation(Exp) for most score tiles; the
    rest are egressed PSUM->SBUF by DVE and exponentiated on GPSIMD
    via tensor_tensor(pow) with base exp(1/512) (scores carry a 512x
    scale from the fp8 power-of-2 scaling).
  - AV: transposed formulation. out[q,65] += ex[k,q]^T v[k,65] per
    (head, 128-query block, key block) -- full 128-partition output
    (the baseline's [65,1024] orientation wasted half the PE). The
    65th ones-lane of v gives the softmax denominator per query ON
    THE PARTITION, so normalization is a per-partition reciprocal +
    tensor_scalar multiply; no cross-partition broadcast needed.
  - head-merge transpose o[q,chan] -> oT[chan,q] via dma_start_transpose
    (DMA xbar, 14ns/32x32 tile), freeing PE and DVE.
  - output projection: bf16 as baseline, f16 store.

The stream is emitted flat in deadline order (baseline-style): scores
for unit u+1 interleave kb-wise with AV of unit u; q/k/v projection
blocks are pinned just before first use; output-projection halves
drip in once their query half's heads are done.
"""

from contextlib import ExitStack

import numpy as np

import concourse.bass as bass
import concourse.tile as tile
from concourse import mybir
from concourse.bass_utils import run_bass_kernel_spmd
from concourse.vector_clock import ScopedClock, VectorClock
from concourse.tile_sem_assignment import N_PROCS

F32 = mybir.dt.float32
F16 = mybir.dt.float16
BF16 = mybir.dt.bfloat16
F8 = mybir.dt.float8e4

B, G, C, H = 4, 2048, 1024, 16
N_CORES = 8
H_LOC = H // 2          # heads per core
O_LOC = H_LOC * 64      # output channels per core
D = 64                  # head dim
CC = C // 128           # contraction blocks for projections
KC = G // 128           # key-token blocks
T = H_LOC // 2          # head pairs per core
NU = 2 * H_LOC          # units: (query-half, head)

SX = 8.0                # x fp8 scale
SW = 64.0               # W fp8 scale
SQK = 2.0 ** -6         # psum(512*q) -> q8 at scale 8
SV = 2.0 ** -9          # psum(512*v) -> v
EXP_SCALE = 1.0 / 512.0  # score psum carries 64x; logits are /8
DR = mybir.MatmulPerfMode.DoubleRow

# kb values (0..15) whose exp runs on GPSIMD via DVE egress + pow
# (interleaved with ScalarE tiles so consecutive sc-ring tiles never
# serialize on one consumer engine)
POOL_KB = {2, 5, 7, 10, 13, 15}
WARM_N = 34             # dummy matmuls bridging the input-DMA front
DRIP_RATE = 0.12


class SplitDrainTileContext(tile.TileContext):
    """Tail drain limited to one sync wait per instruction.

    This environment's walrus rejects >1 sync wait per instruction, so
    wait on each outstanding proc tick with its own NOP first and emit
    the drain bare.
    """

    def _drain_and_barrier(self, tick_clock, wait_clock):
        g = tick_clock.global_clock
        for p in range(N_PROCS):
            if g[p] > 0:
                nop = self.nc.sync.nop(nofuse=True)
                partial = VectorClock([g[q] if q == p else 0 for q in range(N_PROCS)])
                wait_clock.add_sem_waits(nop.ins, ScopedClock({None: partial}))
        self.nc.sync.drain()
        self.nc.all_engine_barrier()
        assert self.sems is not None
        popped = self.nc._tile_sem_poison_stack.pop()
        assert popped is self._sem_poison
        self.nc.clear_and_free_semaphores(list(self.sems.allocated().values()))
        self.nc.all_engine_barrier()


def split_multi_waits(nc):
    """Hoist extra sync waits onto NOPs before each offending instruction
    (this walrus accepts at most one sync wait per instruction)."""
    n_split = 0
    for f in nc.m.functions:
        for bb in f.blocks:
            insts = bb.instructions
            out = []
            for inst in insts:
                si = inst.sync_info
                waits = list(si.on_wait) if si and si.on_wait else []
                if len(waits) > 1:
                    for w in waits[:-1]:
                        nop = mybir.InstNoOp(
                            name=f"{inst.name}_w{n_split}",
                            engine=inst.engine,
                            ins=[],
                            outs=[],
                            sync_info=mybir.SyncInfo(on_wait=[w], on_update=[]),
                        )
                        out.append(nop)
                        n_split += 1
                    inst.sync_info = mybir.SyncInfo(
                        on_wait=[waits[-1]],
                        on_update=list(si.on_update) if si.on_update else [],
                    )
                out.append(inst)
            if len(out) != len(insts):
                bb.instructions[:] = out
    return n_split


def build_program():
    nc = bass.Bass()
    x8_d = nc.declare_dram_parameter("x8", [C, G], F8, isOutput=False)
    xr_d = nc.declare_dram_parameter("xr", [C, G], F8, isOutput=False)
    wq8_d = nc.declare_dram_parameter("wq8", [C, O_LOC], F8, isOutput=False)
    wqr_d = nc.declare_dram_parameter("wqr", [C, O_LOC], F8, isOutput=False)
    wk8_d = nc.declare_dram_parameter("wk8", [C, O_LOC], F8, isOutput=False)
    wkr_d = nc.declare_dram_parameter("wkr", [C, O_LOC], F8, isOutput=False)
    wv8_d = nc.declare_dram_parameter("wv8", [C, O_LOC], F8, isOutput=False)
    wvr_d = nc.declare_dram_parameter("wvr", [C, O_LOC], F8, isOutput=False)
    wp_d = nc.declare_dram_parameter("wp", [O_LOC, C], BF16, isOutput=False)
    out_p = nc.declare_dram_parameter("out_p", [G, C], F16, isOutput=True)

    with SplitDrainTileContext(nc) as tc, ExitStack() as ctx:
        persist = ctx.enter_context(tc.tile_pool(name="persist", bufs=1))
        x8 = persist.tile([128, CC, G], F8, name="x8s", tag="x8s")
        xr = persist.tile([128, CC, G], F8, name="xrs", tag="xrs")
        wq8 = persist.tile([128, CC, O_LOC], F8, name="wq8s", tag="wq8s")
        wqr = persist.tile([128, CC, O_LOC], F8, name="wqrs", tag="wqrs")
        wk8 = persist.tile([128, CC, O_LOC], F8, name="wk8s", tag="wk8s")
        wkr = persist.tile([128, CC, O_LOC], F8, name="wkrs", tag="wkrs")
        wv8 = persist.tile([128, CC, O_LOC], F8, name="wv8s", tag="wv8s")
        wvr = persist.tile([128, CC, O_LOC], F8, name="wvrs", tag="wvrs")
        wp_sb = persist.tile([128, T, C], BF16, name="wp_sb", tag="wp_sb")
        qT8 = [persist.tile([128, 2, G], F8, name=f"qT8_{t}", tag=f"qT8_{t}")
               for t in range(T)]
        kT8 = [persist.tile([128, 2, G], F8, name=f"kT8_{t}", tag=f"kT8_{t}")
               for t in range(T)]
        v_sb = persist.tile([128, KC, H_LOC, 65], BF16, name="v_sb", tag="v_sb")
        oT_t = [persist.tile([128, G], BF16, name=f"oT{t}", tag=f"oT{t}")
                for t in range(T)]
        cpow = persist.tile([128, 1024], F32, name="cpow", tag="cpow")

        # Input loads (one queue, dependency-priority order; transfers
        # serialize on the DMA fabric).
        def ld(dst, src, rearr):
            nc.sync.dma_start(out=dst[:], in_=src.rearrange(rearr, p=128))
        RX = "(cc p) g -> p cc g"
        RW = "(cc p) o -> p cc o"
        nc.sync.dma_start(out=x8[:, :, 0:1024],
                          in_=x8_d[:, 0:1024].rearrange(RX, p=128))
        ld(wq8, wq8_d, RW)
        ld(wqr, wqr_d, RW)
        nc.sync.dma_start(out=xr[:, :, 0:1024],
                          in_=xr_d[:, 0:1024].rearrange(RX, p=128))
        ld(wk8, wk8_d, RW)
        ld(wkr, wkr_d, RW)
        nc.sync.dma_start(out=x8[:, :, 1024:2048],
                          in_=x8_d[:, 1024:2048].rearrange(RX, p=128))
        nc.sync.dma_start(out=xr[:, :, 1024:2048],
                          in_=xr_d[:, 1024:2048].rearrange(RX, p=128))
        ld(wv8, wv8_d, RW)
        ld(wvr, wvr_d, RW)
        ld(wp_sb, wp_d, "(ct p) o -> p ct o")

        # Constants + warm-up (PE clock ramp while DMAs land).
        wmpool = ctx.enter_context(tc.tile_pool(name="warm", bufs=1))
        wz = wmpool.tile([128, 512], BF16, name="wz", tag="wz")
        nc.vector.memset(wz[:], 0.0)
        ones_t = wmpool.tile([128, KC, H_LOC, 1], BF16, name="ones_t", tag="ones")
        nc.vector.memset(ones_t[:], 1.0)
        nc.vector.tensor_copy(out=v_sb[:, :, :, 64:65], in_=ones_t[:])
        nc.vector.memset(cpow[:], float(np.exp(EXP_SCALE)))

        scps = ctx.enter_context(tc.tile_pool(name="sc", bufs=2, space="PSUM"))
        avps = ctx.enter_context(tc.tile_pool(name="avp", bufs=2, space="PSUM"))
        fillps = ctx.enter_context(tc.tile_pool(name="fillp", bufs=2, space="PSUM"))
        expool = ctx.enter_context(tc.tile_pool(name="ex", bufs=24))
        scspool = ctx.enter_context(tc.tile_pool(name="scs", bufs=3))
        opool = ctx.enter_context(tc.tile_pool(name="ostg", bufs=2))
        rpool = ctx.enter_context(tc.tile_pool(name="rcp", bufs=4))
        stpool = ctx.enter_context(tc.tile_pool(name="st", bufs=4))

        def warm(n):
            wps = fillps.tile([128, 512], F32, name="wps", tag="fill")
            for _ in range(n):
                nc.tensor.matmul(wps[:], wz[:, 0:128], wz[:], start=True, stop=True)

        warm(WARM_N)

        written = set()

        def mark(*key):
            written.add(key)

        def need(*key):
            assert key in written, f"use-before-def: {key}"

        # ---- projection building blocks (compensated fp8 DoubleRow) ----
        TERMS_QK = ((0, 0), (0, 1), (1, 0))  # (w residual?, x residual?)

        def v_block(gc):
            ps = fillps.tile([128, 512], F32, name="ps_v", tag="fill")
            first, last = (0, 0, 0), (1, 0, CC // 2 - 1)
            for (rw, rx) in TERMS_QK:
                wv_ = wvr if rw else wv8
                x_ = xr if rx else x8
                for j in range(CC // 2):
                    nc.tensor.matmul(
                        ps[:],
                        x_[:, 2 * j:2 * j + 2, gc * 128:(gc + 1) * 128],
                        wv_[:, 2 * j:2 * j + 2, :],
                        start=((rw, rx, j) == first), stop=((rw, rx, j) == last),
                        perf_mode=DR,
                    )
            nc.scalar.mul(v_sb[:, gc, :, 0:64], ps[:], SV)
            mark("v", gc)

        def qk_quarter(t, which, gh, q4):
            g0 = gh * 1024 + q4 * 256
            ps = fillps.tile([128, 256], F32, name="ps_qk", tag="fill")
            w8_, wr_ = (wq8, wqr) if which == 0 else (wk8, wkr)
            first, last = (0, 0, 0), (1, 0, CC // 2 - 1)
            for (rw, rx) in TERMS_QK:
                w_ = wr_ if rw else w8_
                x_ = xr if rx else x8
                for j in range(CC // 2):
                    nc.tensor.matmul(
                        ps[:],
                        w_[:, 2 * j:2 * j + 2, t * 128:(t + 1) * 128],
                        x_[:, 2 * j:2 * j + 2, g0:g0 + 256],
                        start=((rw, rx, j) == first), stop=((rw, rx, j) == last),
                        perf_mode=DR,
                    )
            dst = qT8[t] if which == 0 else kT8[t]
            if which == 0:
                nc.vector.tensor_scalar_mul(dst[:, 0, g0:g0 + 256], ps[:], SQK)
                # q residual at the same scale: qr = psum*2^-6 - q8
                nc.vector.scalar_tensor_tensor(
                    out=dst[:, 1, g0:g0 + 256], in0=ps[:], scalar=SQK,
                    in1=dst[:, 0, g0:g0 + 256],
                    op0=mybir.AluOpType.mult, op1=mybir.AluOpType.subtract,
                )
            else:
                nc.scalar.mul(dst[:, 0, g0:g0 + 256], ps[:], SQK)
                # k8 duplicated into the second k-tile slot (GPSIMD, SBUF only)
                nc.gpsimd.tensor_copy(out=dst[:, 1, g0:g0 + 256],
                                      in_=dst[:, 0, g0:g0 + 256])
            mark("qk", which, t, gh, q4)

        # ---- attention building blocks ----
        def sc_op(u, kb):
            h, qh = u % H_LOC, u // H_LOC
            t, base = h // 2, (h % 2) * 64
            for q4 in range(4):
                need("qk", 0, t, qh, q4)
            need("qk", 1, t, kb // 8, (kb % 8) // 2)
            sc = scps.tile([128, 1024], F32, name="sc", tag="sc")
            for z in range(2):
                nc.tensor.matmul(
                    sc[:, z * 512:(z + 1) * 512],
                    kT8[t][base:base + D, :, kb * 128:(kb + 1) * 128],
                    qT8[t][base:base + D, :,
                           qh * 1024 + z * 512: qh * 1024 + (z + 1) * 512],
                    start=True, stop=True, perf_mode=DR,
                )
            ex = expool.tile([128, 1024], BF16, name="ex", tag="ex")
            if kb in POOL_KB:
                scs = scspool.tile([128, 1024], F32, name="scs", tag="scs")
                nc.vector.tensor_copy(out=scs[:], in_=sc[:])
                nc.gpsimd.tensor_tensor(out=ex[:], in0=cpow[:], in1=scs[:],
                                        op=mybir.AluOpType.pow)
            else:
                nc.scalar.activation(
                    out=ex[:], in_=sc[:],
                    func=mybir.ActivationFunctionType.Exp, scale=EXP_SCALE,
                )
            return ex

        def av_op(u, kc, pos, av2, ex):
            # pos = index of this kc in the slot's processing order (0..15);
            # psum accumulation is order-independent, so Pool-exp'd kbs are
            # consumed last for extra exp slack.
            h = u % H_LOC
            need("v", kc)
            for qb in range(8):
                nc.tensor.matmul(
                    av2[qb // 4][:, qb % 4, :],
                    ex[:, qb * 128:(qb + 1) * 128],
                    v_sb[:, kc, h, :],
                    start=(pos == 0 and qb % 4 == 0),
                    stop=(pos == KC - 1 and qb % 4 == 3),
                    skip_group_check=True,
                )

        ostg = {}

        def norm_op(u, av2):
            h, qh = u % H_LOC, u // H_LOC
            t, base = h // 2, (h % 2) * 64
            if (t, qh) not in ostg:
                ostg[(t, qh)] = opool.tile([128, 8, 128], BF16,
                                           name="ostg", tag="ostg")
            o = ostg[(t, qh)]
            for half in range(2):
                rcp = rpool.tile([128, 4], F32, name="rcp", tag="rcp")
                nc.vector.reciprocal(out=rcp[:], in_=av2[half][:, :, 64])
                for idx in range(4):
                    qb = half * 4 + idx
                    nc.vector.tensor_scalar(
                        out=o[:, qb, base:base + 64],
                        in0=av2[half][:, idx, 0:64],
                        scalar1=rcp[:, idx:idx + 1], scalar2=None,
                        op0=mybir.AluOpType.mult,
                    )
            mark("o", t, qh)

        def transpose_op(t, qh, qb):
            need("o", t, qh)
            nc.sync.dma_start_transpose(
                oT_t[t][:, qh * 1024 + qb * 128: qh * 1024 + (qb + 1) * 128],
                ostg[(t, qh)][:, qb, :])
            mark("oT", t, qh, qb)

        def proj_half(gc, z):
            qh, qb = gc // 8, gc % 8
            for ct in range(T):
                need("oT", ct, qh, qb)
            po = fillps.tile([128, 512], F32, name="po", tag="fill")
            for ct in range(T):
                nc.tensor.matmul(
                    po[:],
                    oT_t[ct][:, gc * 128:(gc + 1) * 128],
                    wp_sb[:, ct, z * 512:(z + 1) * 512],
                    start=(ct == 0), stop=(ct == T - 1),
                )
            st = stpool.tile([128, 512], F16, name="st", tag="st")
            nc.scalar.copy(out=st[:], in_=po[:])
            eng = (nc.sync, nc.scalar)[(2 * gc + z) % 2]
            eng.dma_start(
                out=out_p[gc * 128:(gc + 1) * 128, z * 512:(z + 1) * 512],
                in_=st[:],
            )

        # ---- the flat stream ----
        # Absolute step s: sc(u, kb) at s = 16u + kb; av(u, kc) at
        # s = 16(u+1) + kc; norm(u) at the end of step 16(u+1)+15.
        pinned = {}

        def pin(step, fn):
            pinned.setdefault(max(0, step), []).append(fn)

        # q quarters: needed by sc(u = qh*8 + 2t + (0|1), kb=0).
        for t in range(T):
            for qh in range(2):
                for q4 in range(4):
                    pin(16 * (qh * 8 + 2 * t) - 8 + q4,
                        lambda t=t, qh=qh, q4=q4: qk_quarter(t, 0, qh, q4))
        # k quarters: quarter (gh, q4) covers kb = gh*8 + 2*q4 (+1); first
        # used by unit 2t at s = 32t + kb.
        for t in range(T):
            for gh in range(2):
                for q4 in range(4):
                    pin(32 * t + gh * 8 + 2 * q4 - 6,
                        lambda t=t, gh=gh, q4=q4: qk_quarter(t, 1, gh, q4))
        # v blocks: v(kc) needed by av(0, kc) at s = 16 + pos(kc) in the
        # reordered AV consumption sequence.
        AV_ORDER0 = [kb for kb in range(KC) if kb not in POOL_KB] + sorted(POOL_KB)
        for kc in range(KC):
            pin(14 + AV_ORDER0.index(kc), lambda kc=kc: v_block(kc))
        # proj halves for query-half 0: transposes done after norm(u=7)
        # (end of s=143); drip them across slots 9..16.
        for i, (gc, z) in enumerate((gc, z) for gc in range(8) for z in range(2)):
            pin(150 + 7 * i, lambda gc=gc, z=z: proj_half(gc, z))

        filler = []
        filler_i = 0
        filler_budget = 0.0

        def drip(budget):
            nonlocal filler_i, filler_budget
            if filler_i >= len(filler):
                filler_budget = 0.0
                return
            filler_budget = min(filler_budget + budget, 2.0)
            while filler_budget >= 1.0 and filler_i < len(filler):
                gc, z = filler[filler_i]
                filler_i += 1
                filler_budget -= 1.0
                proj_half(gc, z)

        exs = {}
        av_tiles = {}

        def issue_sc(u, kb):
            exs[(u, kb)] = sc_op(u, kb)

        # AV consumption order: ScalarE-exp'd kbs first, Pool-exp'd last.
        AV_ORDER = [kb for kb in range(KC) if kb not in POOL_KB] + sorted(POOL_KB)

        n_steps = 16 * (NU + 1)
        for s in range(n_steps):
            slot, kb = divmod(s, 16)
            u_sc = slot            # scores unit (one slot ahead)
            u_av = slot - 1        # AV unit
            for op in pinned.pop(s, ()):
                op()
            if u_av >= 0:
                if kb == 0:
                    av_tiles[u_av] = [
                        avps.tile([128, 4, 65], F32, name="av", tag="av")
                        for _ in range(2)]
                kc = AV_ORDER[kb]
                av_op(u_av, kc, kb, av_tiles[u_av], exs.pop((u_av, kc)))
                if s >= 160:
                    drip(DRIP_RATE)
            if u_sc < NU:
                issue_sc(u_sc, kb)
            if u_av >= 0 and kb == 15:
                norm_op(u_av, av_tiles.pop(u_av))
                h, qh = u_av % H_LOC, u_av // H_LOC
                if h % 2 == 1:
                    for qb in range(8):
                        transpose_op(h // 2, qh, qb)
                if u_av == NU - 1:
                    filler.extend((gc, z) for gc in range(8, KC)
                                  for z in range(2))
        for op_step in sorted(pinned):
            for op in pinned.pop(op_step, ()):
                op()
        while filler_i < len(filler):
            gc, z = filler[filler_i]
            filler_i += 1
            proj_half(gc, z)

    split_multi_waits(nc)
    return nc


_CACHE = {}

_F8_NP = mybir.dt.np(F8)
_BF16_NP = mybir.dt.np(BF16)


def _split_f8(a, scale):
    hi = (a * scale).astype(_F8_NP)
    lo = (a * scale - hi.astype(np.float32)).astype(_F8_NP)
    return hi, lo


def make_in_maps(x, Wq, Wk, Wv, Wp):
    x = np.asarray(x, dtype=np.float32)
    WqT = np.asarray(Wq, dtype=np.float32).T
    WkT = np.asarray(Wk, dtype=np.float32).T
    WvT = np.asarray(Wv, dtype=np.float32).T
    WpT = np.asarray(Wp, dtype=np.float32).T
    in_maps = []
    for core in range(N_CORES):
        b, s = core // 2, core % 2
        osl = slice(s * O_LOC, (s + 1) * O_LOC)
        x8, xr = _split_f8(np.ascontiguousarray(x[b].T), SX)
        wq8, wqr = _split_f8(np.ascontiguousarray(WqT[:, osl]), SW)
        wk8, wkr = _split_f8(np.ascontiguousarray(WkT[:, osl]), SW)
        wv8, wvr = _split_f8(np.ascontiguousarray(WvT[:, osl]), SW)
        in_maps.append({
            "x8": x8, "xr": xr,
            "wq8": wq8, "wqr": wqr,
            "wk8": wk8, "wkr": wkr,
            "wv8": wv8, "wvr": wvr,
            "wp": np.ascontiguousarray(WpT[osl, :]).astype(_BF16_NP),
        })
    return in_maps


def kernel(x, Wq, Wk, Wv, Wp, bp):
    in_maps = make_in_maps(x, Wq, Wk, Wv, Wp)
    if "nc" not in _CACHE:
        _CACHE["nc"] = build_program()
    res = run_bass_kernel_spmd(_CACHE["nc"], in_maps, list(range(N_CORES)))
    out = np.zeros((B, G, C), np.float32)
    bp = np.asarray(bp, dtype=np.float32)
    for b in range(B):
        out[b] = (res.results[2 * b]["out_p"].astype(np.float32)
                  + res.results[2 * b + 1]["out_p"].astype(np.float32) + bp)
    return out
